# revision 34
# baseline (speedup 1.0000x reference)
"""HGCN (hypergraph conv net) Trainium2 kernel, 8-core SPMD.

Strategy:
  - Graph-aligned node sharding: core c owns graphs [8c, 8c+8). Node rows and
    hyperedge slots are load-balanced (greedy LPT on per-core incidence
    counts) so every 128-row block needs the same minimal number of 128-entry
    gather chunks on every core (the per-core program is uniform SPMD while
    per-core data differs).
  - Reformulation: e = Binv * (H^T h) @ W  (aggregate raw h into hyperedges
    BEFORE the dense transform). Each core aggregates only its own nodes'
    incidence entries, so the only cross-core exchange per layer is
    ReduceScatter(u_partial) + AllGather(e) on the small hyperedge tensor;
    the W transform runs on each core's tiny ReduceScatter shard.
  - The hedge dimension is split into two halves pipelined end-to-end:
    RS(half0) fires mid phase A; its transform+AllGather overlap phase A's
    tail; phase C's node-side gathers are split per hedge half (exact-sliced
    gather sources), so half 0's aggregation overlaps RS/transform/AG of
    half 1. A bf16 SBUF accumulator holds each node block's half-0 partial
    (with conv bias folded in); half 1 adds it back via an identity matmul
    into the open PSUM accumulation, keeping the Vector engine light.
  - Segment sums are computed as one-hot matmuls on the PE: dma_gather pulls
    sorted entries' rows onto the 128 partitions, a host-built one-hot
    matrix (with Binv/Dinv degree scaling folded in) is the stationary
    operand, and PSUM accumulates chunks of a 128-hedge/128-node block.
  - LayerNorm stats via fused bn_stats/bn_aggr straight out of PSUM.
  - Pooling (mean via per-block one-hot matmuls summed in SBUF, max via PE
    transpose + running VE max) is fused into the last layer's node-side
    pass, so the final h never round-trips DRAM.
  - bf16 data path, f32 accumulation/statistics.
"""

import math
import os

import numpy as np
from ml_dtypes import bfloat16

import concourse.bacc as bacc
import concourse.bass as bass
import concourse.mybir as mybir
import concourse.tile as tile
from concourse import masks
from concourse.bass_utils import run_bass_kernel_spmd

# ---------------------------------------------------------------- constants
NCORES = 8
N_NODES = 50000
N_INC = 300000
N_HE = 10000
NG = 64
IN_C = 768
HID = 512
NL = 3
NCLS = 2

P = 128
GPC = NG // NCORES            # graphs per core
NHB = 80                      # hedge blocks of 128 -> HE_PAD rows
HE_PAD = NHB * P              # 10240
HB_BLKS = NHB // 2            # blocks per hedge half
HALF = HB_BLKS * P            # 5120 hedges per half
SB_CH = 24                    # gather-batch size in chunks
LN_EPS = 1e-5
NEG = -1.0e30

f32 = mybir.dt.float32
bf16 = mybir.dt.bfloat16
i16 = mybir.dt.int16
AF = mybir.ActivationFunctionType
ALU = mybir.AluOpType


# ---------------------------------------------------------------- host prep
def _wrap_idx(idx, nch):
    """dma_gather index layout: idx j -> [j%16, j//16], tiled to 128 parts."""
    cols = nch * 8
    w = np.zeros((16, cols), np.int16)
    w[np.arange(idx.size) % 16, np.arange(idx.size) // 16] = idx.astype(np.int16)
    return np.tile(w, (8, 1))


def _pm(a):
    """[NB*128, F] row-major blocks -> partition-major [128, NB*F]."""
    nb = a.shape[0] // P
    return np.ascontiguousarray(
        a.reshape(nb, P, a.shape[1]).transpose(1, 0, 2).reshape(P, nb * a.shape[1])
    )


def _side_schedule(block_of_entry, n_blocks):
    """Uniform chunk schedule: chunks per block = max over cores, min 1."""
    counts = np.zeros((NCORES, n_blocks), np.int64)
    for c in range(NCORES):
        blk = block_of_entry[c]
        np.add.at(counts[c], blk, 1)
    chunks = np.maximum(1, -(-counts.max(axis=0) // P))  # ceil
    chunk_map = []  # (block, is_first, is_last)
    for k in range(n_blocks):
        for j in range(chunks[k]):
            chunk_map.append((k, j == 0, j == chunks[k] - 1))
    return chunks, chunk_map


def _batches(n_chunks):
    out = []
    s = 0
    while s < n_chunks:
        n = min(SB_CH, n_chunks - s)
        out.append((s, n))
        s += n
    return out


def _node_side(e_lrow, e_slot, nnb, Dinv_l, slot_lo, slot_hi, rebase):
    """Build node-side gather idx + one-hot scatter for entries whose hedge
    slot is in [slot_lo, slot_hi). Gather idx rebased by -rebase."""
    n_chunks, n_map = _side_schedule(
        [el[(es >= slot_lo) & (es < slot_hi)] // P
         for el, es in zip(e_lrow, e_slot)],
        nnb,
    )
    ch_n = len(n_map)
    chunk_base = np.concatenate([[0], np.cumsum(n_chunks)])
    per_core = []
    for c in range(NCORES):
        sel = (e_slot[c] >= slot_lo) & (e_slot[c] < slot_hi)
        el = e_lrow[c][sel]
        es = e_slot[c][sel]
        order = np.argsort(el, kind="stable")
        el = el[order]
        es = es[order]
        blk = el // P
        gin = np.zeros(ch_n * P, np.int64)
        sn = np.zeros((ch_n, P, P), np.float32)
        for b in range(nnb):
            m = blk == b
            n = int(m.sum())
            if n == 0:
                continue
            base = chunk_base[b] * P
            pos = base + np.arange(n)
            gin[pos] = es[m] - rebase
            sn[pos // P, pos % P, el[m] - b * P] = Dinv_l[c, el[m]]
        per_core.append(
            (
                _pm(sn.reshape(ch_n * P, P)).astype(bfloat16),
                _wrap_idx(gin, ch_n),
            )
        )
    return ch_n, n_map, per_core


def preprocess(inputs):
    x = np.asarray(inputs["x"], np.float32)
    node_idx = np.asarray(inputs["node_idx"]).astype(np.int64)
    hedge_idx = np.asarray(inputs["hedge_idx"]).astype(np.int64)
    batch = np.asarray(inputs["batch"]).astype(np.int64)

    cnt_g = np.bincount(batch, minlength=NG)
    gslot = max(896, -(-int(cnt_g.max()) // P) * P)
    npcp = GPC * gslot                      # local (padded) rows per core
    nnb = npcp // P                         # node blocks per core
    nbins = gslot // P                      # node blocks per graph slot

    core_of_node = batch // GPC
    ecore = core_of_node[node_idx]

    D = np.bincount(node_idx, minlength=N_NODES)
    B = np.bincount(hedge_idx, minlength=N_HE)
    Dinv = np.where(D > 0, 1.0 / np.maximum(D, 1), 0.0).astype(np.float32)
    Binv = np.where(B > 0, 1.0 / np.maximum(B, 1), 0.0).astype(np.float32)

    # ---- hedge -> slot balancing: equalize per-(core, block) entry counts
    cnt8 = np.zeros((N_HE, NCORES), np.int64)
    np.add.at(cnt8, (hedge_idx, ecore), 1)
    binsum = np.zeros((NHB, NCORES), np.int64)
    binn = np.zeros(NHB, np.int64)
    slot_of = np.zeros(N_HE, np.int64)
    for h in np.argsort(-cnt8.sum(1), kind="stable"):
        score = (binsum + cnt8[h]).max(axis=1).astype(np.float64)
        score[binn >= P] = 1e18
        j = int(np.argmin(score))
        slot_of[h] = j * P + binn[j]
        binn[j] += 1
        binsum[j] += cnt8[h]
    Binv_s = np.zeros(HE_PAD, np.float32)
    Binv_s[slot_of] = Binv
    e_slot_all = slot_of[hedge_idx]

    # ---- node -> local row balancing (per core, per graph; balance both
    #      hedge halves' entry counts across the graph's blocks)
    d0 = np.zeros(N_NODES, np.int64)
    d1 = np.zeros(N_NODES, np.int64)
    np.add.at(d0, node_idx[e_slot_all < HALF], 1)
    np.add.at(d1, node_idx[e_slot_all >= HALF], 1)
    lrow = np.zeros(N_NODES, np.int64)
    for gg in range(NG):
        nodes = np.nonzero(batch == gg)[0]
        g = gg % GPC
        dd0 = d0[nodes]
        dd1 = d1[nodes]
        bs0 = np.zeros(nbins, np.int64)
        bs1 = np.zeros(nbins, np.int64)
        bn = np.zeros(nbins, np.int64)
        for i in np.argsort(-(dd0 + dd1), kind="stable"):
            sc = np.maximum(bs0 + dd0[i], bs1 + dd1[i]).astype(np.float64)
            sc[bn >= P] = 1e18
            j = int(np.argmin(sc))
            lrow[nodes[i]] = g * gslot + j * P + bn[j]
            bn[j] += 1
            bs0[j] += dd0[i]
            bs1[j] += dd1[i]

    # Dinv in per-core local-row layout
    Dinv_l = np.zeros((NCORES, npcp), np.float32)
    Dinv_l[core_of_node, lrow] = Dinv

    # ---- per-core entry lists (hedge ids already slot-mapped)
    e_lrow, e_slot = [], []
    for c in range(NCORES):
        sel = ecore == c
        e_lrow.append(lrow[node_idx[sel]])
        e_slot.append(e_slot_all[sel])

    # ---- hedge-side schedule (blocks of 128 hedge slots)
    h_chunks, h_map = _side_schedule([es // P for es in e_slot], NHB)
    ch_h = len(h_map)
    chunk_base_h = np.concatenate([[0], np.cumsum(h_chunks)])

    # ---- node-side schedules, split by hedge half
    ch_n0, n_map0, ns0 = _node_side(e_lrow, e_slot, nnb, Dinv_l, 0, HALF, 0)
    ch_n1, n_map1, ns1 = _node_side(
        e_lrow, e_slot, nnb, Dinv_l, HALF, HE_PAD, HALF
    )

    per_core = []
    for c in range(NCORES):
        # hedge side: gather h rows by local node row, scatter to hedge slot
        gih = np.zeros(ch_h * P, np.int64)
        sh = np.zeros((ch_h, P, P), np.float32)
        order = np.argsort(e_slot[c], kind="stable")
        es = e_slot[c][order]
        el = e_lrow[c][order]
        blk = es // P
        for k in range(NHB):
            m = blk == k
            n = int(m.sum())
            if n == 0:
                continue
            base = chunk_base_h[k] * P
            pos = base + np.arange(n)
            gih[pos] = el[m]
            sh[pos // P, pos % P, es[m] - k * P] = Binv_s[es[m]]

        # x in local layout, tiled per (block, k-chunk): [128, nnb*768]
        xl = np.zeros((npcp, IN_C), np.float32)
        nodes_c = np.nonzero(core_of_node == np.int64(c))[0]
        xl[lrow[nodes_c]] = x[nodes_c]
        nkc = IN_C // P
        xkm = np.ascontiguousarray(
            xl.reshape(nnb, P, nkc, P).transpose(3, 0, 2, 1).reshape(P, nnb * IN_C)
        ).astype(bfloat16)

        # pooling one-hot (mean) and masks
        pp = np.zeros((npcp, GPC), np.float32)
        gmask = np.zeros((P, GPC), np.float32)
        maskcol = np.full((npcp, 1), NEG, np.float32)
        for g in range(GPC):
            gg = c * GPC + g
            n = int(cnt_g[gg])
            if n == 0:
                continue
            rows = lrow[np.nonzero(batch == gg)[0]]
            pp[rows, g] = 1.0 / n
            gmask[:, g] = 1.0
            maskcol[rows] = 0.0

        per_core.append(
            dict(
                xkm=xkm,
                S_h=_pm(sh.reshape(ch_h * P, P)).astype(bfloat16),
                idx_h=_wrap_idx(gih, ch_h),
                S_n0=ns0[c][0],
                idx_n0=ns0[c][1],
                S_n1=ns1[c][0],
                idx_n1=ns1[c][1],
                P_pm=_pm(pp).astype(bfloat16),
                maskcol_pm=_pm(maskcol),
                gmask=gmask,
            )
        )

    # ---- shared weights
    bcast = lambda v: np.ascontiguousarray(
        np.broadcast_to(np.asarray(v, np.float32), (P, HID))
    )
    shared = dict(
        Win=np.asarray(inputs["W_in"], np.float32).astype(bfloat16),
        Wc=np.asarray(inputs["conv_W"], np.float32)
        .reshape(NL * HID, HID)
        .astype(bfloat16),
        binb=bcast(inputs["b_in"]),
        convb=np.concatenate([bcast(np.asarray(inputs["conv_b"])[i]) for i in range(NL)]),
        lng=np.concatenate([bcast(np.asarray(inputs["ln_g"])[i]) for i in range(NL)]),
        lnb=np.concatenate([bcast(np.asarray(inputs["ln_b"])[i]) for i in range(NL)]),
        Wp0=np.asarray(inputs["W_p0"], np.float32).astype(bfloat16),
        Wp1=np.asarray(inputs["W_p1"], np.float32).astype(bfloat16),
        Wc0=np.asarray(inputs["W_c0"], np.float32).astype(bfloat16),
        Wc1=np.asarray(inputs["W_c1"], np.float32).astype(bfloat16),
        bp0T=np.ascontiguousarray(
            np.asarray(inputs["b_p0"], np.float32).reshape(4, P).T
        ),
        bp1T=np.ascontiguousarray(
            np.asarray(inputs["b_p1"], np.float32).reshape(2, P).T
        ),
        bc0T=np.ascontiguousarray(
            np.asarray(inputs["b_c0"], np.float32).reshape(1, P).T
        ),
        bc1=np.asarray(inputs["b_c1"], np.float32).reshape(NCLS, 1),
    )

    sched = dict(
        gslot=gslot,
        npcp=npcp,
        nnb=nnb,
        ch_h=ch_h,
        h_map=h_map,
        ch_n0=ch_n0,
        n_map0=n_map0,
        ch_n1=ch_n1,
        n_map1=n_map1,
    )
    return sched, shared, per_core


# ---------------------------------------------------------------- builder
def build(sched, n_cores=NCORES):
    npcp = sched["npcp"]
    nnb = sched["nnb"]
    ch_h = sched["ch_h"]
    ch_n0 = sched["ch_n0"]
    ch_n1 = sched["ch_n1"]
    hsh = HALF // n_cores                 # per-core shard rows per half (640)
    rg = [list(range(n_cores))]

    nc = bacc.Bacc("TRN2", target_bir_lowering=False, debug=False, num_devices=n_cores)

    def inp(name, shape, dt):
        return nc.dram_tensor(name, shape, dt, kind="ExternalInput").ap()

    xkm = inp("xkm", [P, nnb * IN_C], bf16)
    S_h = inp("S_h", [P, ch_h * P], bf16)
    idx_h = inp("idx_h", [P, ch_h * 8], i16)
    S_n0 = inp("S_n0", [P, ch_n0 * P], bf16)
    idx_n0 = inp("idx_n0", [P, ch_n0 * 8], i16)
    S_n1 = inp("S_n1", [P, ch_n1 * P], bf16)
    idx_n1 = inp("idx_n1", [P, ch_n1 * 8], i16)
    P_pm = inp("P_pm", [P, nnb * GPC], bf16)
    maskcol_pm = inp("maskcol_pm", [P, nnb], f32)
    gmask = inp("gmask", [P, GPC], f32)
    Win = inp("Win", [IN_C, HID], bf16)
    Wc = inp("Wc", [NL * HID, HID], bf16)
    binb = inp("binb", [P, HID], f32)
    convb = inp("convb", [NL * P, HID], f32)
    lng = inp("lng", [NL * P, HID], f32)
    lnb = inp("lnb", [NL * P, HID], f32)
    Wp0 = inp("Wp0", [2 * HID, HID], bf16)
    Wp1 = inp("Wp1", [HID, HID // 2], bf16)
    Wc0 = inp("Wc0", [HID // 2, HID // 4], bf16)
    Wc1 = inp("Wc1", [HID // 4, NCLS], bf16)
    bp0T = inp("bp0T", [P, 4], f32)
    bp1T = inp("bp1T", [P, 2], f32)
    bc0T = inp("bc0T", [P, 1], f32)
    bc1 = inp("bc1", [NCLS, 1], f32)

    out = nc.dram_tensor("out", [NCLS, GPC], f32, kind="ExternalOutput").ap()

    h0 = nc.dram_tensor("h0", [npcp, HID], bf16).ap()
    hA = nc.dram_tensor("hA", [npcp, HID], bf16).ap()
    hB = nc.dram_tensor("hB", [npcp, HID], bf16).ap()
    u_q = [nc.dram_tensor(f"u_part{q}", [HALF, HID], bf16).ap() for q in range(2)]
    urs_q = [nc.dram_tensor(f"u_rs{q}", [hsh, HID], bf16).ap() for q in range(2)]
    el_q = [nc.dram_tensor(f"e_loc{q}", [hsh, HID], bf16).ap() for q in range(2)]
    e_full = nc.dram_tensor("e_full", [HE_PAD, HID], bf16, addr_space="Shared").ap()

    h_src = [h0, hA, hB]      # phase-A gather source per layer
    h_dst = [hA, hB, None]    # DRAM copy of layer output

    with tile.TileContext(nc) as tc:
        with (
            tc.tile_pool(name="persist", bufs=1) as pers,
            tc.tile_pool(name="psum", bufs=3, space="PSUM") as pp,
            tc.tile_pool(name="psum_sm", bufs=2, space="PSUM") as ptr,
            tc.tile_pool(name="work", bufs=2) as wk,
            tc.tile_pool(name="wconst", bufs=1) as wkc,
            tc.tile_pool(name="gath", bufs=2) as gpg,
            tc.tile_pool(name="gath_s", bufs=2) as gps,
            tc.tile_pool(name="uT", bufs=2) as utp,
            tc.tile_pool(name="stats", bufs=4) as stp,
        ):
            # ---- persistent SBUF
            ixh = pers.tile([P, ch_h * 8], i16, tag="ixh")
            nc.sync.dma_start(out=ixh[:], in_=idx_h[:])
            ixn0 = pers.tile([P, ch_n0 * 8], i16, tag="ixn0")
            nc.sync.dma_start(out=ixn0[:], in_=idx_n0[:])
            ixn1 = pers.tile([P, ch_n1 * 8], i16, tag="ixn1")
            nc.sync.dma_start(out=ixn1[:], in_=idx_n1[:])
            epst = pers.tile([P, 1], f32, tag="eps")
            nc.vector.memset(epst[:], LN_EPS)
            ident = pers.tile([P, P], bf16, tag="ident")
            masks.make_identity(nc, ident[:])
            t_half = pers.tile([P, nnb * HID], bf16, tag="t_half")
            mean_sb = pers.tile([P, 4 * GPC], f32, tag="mean_sb")
            nc.vector.memset(mean_sb[:], 0.0)
            gmax_sb = pers.tile([P, 4 * GPC], f32, tag="gmax")
            nc.vector.memset(gmax_sb[:], NEG)

            # ================= input projection =================
            with tc.tile_pool(name="inproj", bufs=1) as ip, tc.tile_pool(
                name="inproj_x", bufs=3
            ) as ipx:
                nkc = IN_C // P
                wts = []
                for kc in range(nkc):
                    t = ip.tile([P, HID], bf16, tag=f"win{kc}")
                    nc.sync.dma_start(out=t[:], in_=Win[kc * P : (kc + 1) * P, :])
                    wts.append(t)
                binb_t = ip.tile([P, HID], f32, tag="binb")
                nc.sync.dma_start(out=binb_t[:], in_=binb[:])

                for b in range(nnb):
                    xt = ipx.tile([P, IN_C], bf16, tag="xkm")
                    nc.sync.dma_start(
                        out=xt[:], in_=xkm[:, b * IN_C : (b + 1) * IN_C]
                    )
                    ps = pp.tile([P, HID], f32, tag="mm")
                    for kc in range(nkc):
                        nc.tensor.matmul(
                            out=ps[:],
                            lhsT=xt[:, kc * P : (kc + 1) * P],
                            rhs=wts[kc][:],
                            start=(kc == 0),
                            stop=(kc == nkc - 1),
                        )
                    t = wk.tile([P, HID], f32, tag="ip_t")
                    nc.vector.tensor_add(t[:], ps[:], binb_t[:])
                    ht = wk.tile([P, HID], bf16, tag="ip_h")
                    nc.scalar.activation(ht[:], t[:], AF.Relu)
                    nc.sync.dma_start(out=h0[b * P : (b + 1) * P, :], in_=ht[:])

            # ================= conv layers =================
            for li in range(NL):
                h_in = h_src[li]
                last = li == NL - 1

                convb_t = wkc.tile([P, HID], f32, tag="convb")
                nc.sync.dma_start(out=convb_t[:], in_=convb[li * P : (li + 1) * P, :])
                lng_t = wkc.tile([P, HID], f32, tag="lng")
                nc.sync.dma_start(out=lng_t[:], in_=lng[li * P : (li + 1) * P, :])
                lnb_t = wkc.tile([P, HID], f32, tag="lnb")
                nc.sync.dma_start(out=lnb_t[:], in_=lnb[li * P : (li + 1) * P, :])
                wcs = []
                for kc in range(4):
                    t = wkc.tile([P, HID], bf16, tag=f"wc{kc}")
                    nc.sync.dma_start(
                        out=t[:], in_=Wc[li * HID + kc * P : li * HID + (kc + 1) * P, :]
                    )
                    wcs.append(t)
                if last:
                    mask_t = wkc.tile([P, nnb], f32, tag="maskc")
                    nc.sync.dma_start(out=mask_t[:], in_=maskcol_pm[:])
                    ppool_t = wkc.tile([P, nnb * GPC], bf16, tag="Ppm")
                    nc.sync.dma_start(out=ppool_t[:], in_=P_pm[:])

                def emit_RS(q):
                    nc.gpsimd.collective_compute(
                        "ReduceScatter",
                        ALU.add,
                        replica_groups=rg,
                        ins=[u_q[q][:]],
                        outs=[urs_q[q][:]],
                    )

                def emit_B(q):
                    # transform this core's shard: e = u_rs @ W
                    uts = []
                    for fc in range(4):
                        t = utp.tile([P, hsh], bf16, tag=f"uT{fc}")
                        nc.sync.dma_start(
                            out=t[:],
                            in_=urs_q[q][:, fc * P : (fc + 1) * P],
                            transpose=True,
                        )
                        uts.append(t)
                    for rt in range(hsh // P):
                        ps = pp.tile([P, HID], f32, tag="mm")
                        for kc in range(4):
                            nc.tensor.matmul(
                                out=ps[:],
                                lhsT=uts[kc][:, rt * P : (rt + 1) * P],
                                rhs=wcs[kc][:],
                                start=(kc == 0),
                                stop=(kc == 3),
                            )
                        eb = wk.tile([P, HID], bf16, tag="e_bf")
                        nc.scalar.copy(eb[:], ps[:])
                        nc.sync.dma_start(
                            out=el_q[q][rt * P : (rt + 1) * P, :], in_=eb[:]
                        )

                def emit_AG(q):
                    nc.gpsimd.collective_compute(
                        "AllGather",
                        ALU.bypass,
                        replica_groups=rg,
                        ins=[el_q[q][:]],
                        outs=[e_full[q * HALF : (q + 1) * HALF, :]],
                    )

                # ---------- phase A: hedge-side aggregation ----------
                b0_at = None
                ag0_at = None
                cur_ps = None
                batches = _batches(ch_h)
                for bi, (c0, nch) in enumerate(batches):
                    if b0_at is not None and bi >= b0_at:
                        emit_B(0)
                        b0_at = None
                    if ag0_at is not None and bi >= ag0_at:
                        emit_AG(0)
                        ag0_at = None
                    gt = gpg.tile([P, SB_CH * HID], bf16, tag="gt")
                    nc.gpsimd.dma_gather(
                        out_ap=gt[:, : nch * HID].rearrange(
                            "p (c f) -> p c f", f=HID
                        ),
                        in_ap=h_in[:, :],
                        idxs_ap=ixh[:, c0 * 8 : (c0 + nch) * 8],
                        num_idxs=nch * P,
                        num_idxs_reg=nch * P,
                        elem_size=HID,
                        single_packet=False,
                    )
                    st = gps.tile([P, SB_CH * P], bf16, tag="st")
                    nc.sync.dma_start(
                        out=st[:, : nch * P], in_=S_h[:, c0 * P : (c0 + nch) * P]
                    )
                    gt3 = gt[:, : nch * HID].rearrange("p (c f) -> p c f", f=HID)
                    for ci in range(nch):
                        k, is_first, is_last = sched["h_map"][c0 + ci]
                        if is_first:
                            cur_ps = pp.tile([P, HID], f32, tag="mm")
                        nc.tensor.matmul(
                            out=cur_ps[:],
                            lhsT=st[:, ci * P : (ci + 1) * P],
                            rhs=gt3[:, ci, :],
                            start=is_first,
                            stop=is_last,
                        )
                        if not is_last:
                            continue
                        ub = wk.tile([P, HID], bf16, tag="u_bf")
                        nc.scalar.copy(ub[:], cur_ps[:])
                        q = k // HB_BLKS
                        kl = k - q * HB_BLKS
                        nc.sync.dma_start(
                            out=u_q[q][kl * P : (kl + 1) * P, :], in_=ub[:]
                        )
                        if k == HB_BLKS - 1:
                            emit_RS(0)
                            b0_at = bi + 4
                            ag0_at = bi + 5
                        elif k == NHB - 1:
                            emit_RS(1)
                if b0_at is not None:
                    emit_B(0)
                if ag0_at is not None:
                    emit_AG(0)

                # ---------- phase C half 0 ----------
                # B(1)+AG(1) are emitted a few half-0 gather batches in, so
                # their dep waits (RS(1) completion) never stall the queues.
                cur_ps = None
                b1_at, ag1_at = 4, 5
                for bi, (c0, nch) in enumerate(_batches(ch_n0)):
                    if b1_at is not None and bi >= b1_at:
                        emit_B(1)
                        b1_at = None
                    if ag1_at is not None and bi >= ag1_at:
                        emit_AG(1)
                        ag1_at = None
                    gt = gpg.tile([P, SB_CH * HID], bf16, tag="gt")
                    nc.gpsimd.dma_gather(
                        out_ap=gt[:, : nch * HID].rearrange(
                            "p (c f) -> p c f", f=HID
                        ),
                        in_ap=e_full[0:HALF, :],
                        idxs_ap=ixn0[:, c0 * 8 : (c0 + nch) * 8],
                        num_idxs=nch * P,
                        num_idxs_reg=nch * P,
                        elem_size=HID,
                        single_packet=False,
                    )
                    st = gps.tile([P, SB_CH * P], bf16, tag="st")
                    nc.sync.dma_start(
                        out=st[:, : nch * P], in_=S_n0[:, c0 * P : (c0 + nch) * P]
                    )
                    gt3 = gt[:, : nch * HID].rearrange("p (c f) -> p c f", f=HID)
                    for ci in range(nch):
                        b, is_first, is_last = sched["n_map0"][c0 + ci]
                        if is_first:
                            cur_ps = pp.tile([P, HID], f32, tag="mm")
                        nc.tensor.matmul(
                            out=cur_ps[:],
                            lhsT=st[:, ci * P : (ci + 1) * P],
                            rhs=gt3[:, ci, :],
                            start=is_first,
                            stop=is_last,
                        )
                        if is_last:
                            # t_half = half0 aggregate + conv bias
                            nc.vector.tensor_add(
                                t_half[:, b * HID : (b + 1) * HID],
                                cur_ps[:],
                                convb_t[:],
                            )
                if b1_at is not None:
                    emit_B(1)
                if ag1_at is not None:
                    emit_AG(1)

                # ---------- phase C half 1 + LN ----------
                cur_ps = None
                for (c0, nch) in _batches(ch_n1):
                    gt = gpg.tile([P, SB_CH * HID], bf16, tag="gt")
                    nc.gpsimd.dma_gather(
                        out_ap=gt[:, : nch * HID].rearrange(
                            "p (c f) -> p c f", f=HID
                        ),
                        in_ap=e_full[HALF:HE_PAD, :],
                        idxs_ap=ixn1[:, c0 * 8 : (c0 + nch) * 8],
                        num_idxs=nch * P,
                        num_idxs_reg=nch * P,
                        elem_size=HID,
                        single_packet=False,
                    )
                    st = gps.tile([P, SB_CH * P], bf16, tag="st")
                    nc.sync.dma_start(
                        out=st[:, : nch * P], in_=S_n1[:, c0 * P : (c0 + nch) * P]
                    )
                    gt3 = gt[:, : nch * HID].rearrange("p (c f) -> p c f", f=HID)
                    for ci in range(nch):
                        b, is_first, is_last = sched["n_map1"][c0 + ci]
                        if is_first:
                            cur_ps = pp.tile([P, HID], f32, tag="mm")
                        nc.tensor.matmul(
                            out=cur_ps[:],
                            lhsT=st[:, ci * P : (ci + 1) * P],
                            rhs=gt3[:, ci, :],
                            start=is_first,
                            stop=False,
                        )
                        if not is_last:
                            continue
                        # add half-0 partial (incl conv bias) on the PE
                        nc.tensor.matmul(
                            out=cur_ps[:],
                            lhsT=ident[:],
                            rhs=t_half[:, b * HID : (b + 1) * HID],
                            start=False,
                            stop=True,
                        )

                        # ---- post-ops for node block b (t stays in PSUM)
                        stats = stp.tile([P, 6], f32, tag="bns")
                        nc.vector.bn_stats(out=stats[:], in_=cur_ps[:])
                        mv = stp.tile([P, 2], f32, tag="mv")
                        nc.vector.bn_aggr(out=mv[:], in_=stats[:])
                        std = stp.tile([P, 1], f32, tag="std")
                        nc.scalar.activation(
                            std[:], mv[:, 1:2], AF.Sqrt, bias=epst[:, 0:1]
                        )
                        rstd = stp.tile([P, 1], f32, tag="rstd")
                        nc.vector.reciprocal(rstd[:], std[:])
                        nmr = stp.tile([P, 1], f32, tag="nmr")
                        nc.vector.tensor_tensor(
                            out=nmr[:], in0=mv[:, 0:1], in1=rstd[:], op=ALU.mult
                        )
                        nc.vector.tensor_scalar_mul(nmr[:], nmr[:], -1.0)
                        xn = wk.tile([P, HID], f32, tag="poxn")
                        nc.scalar.activation(
                            xn[:], cur_ps[:], AF.Identity,
                            bias=nmr[:, 0:1], scale=rstd[:, 0:1],
                        )
                        nc.vector.tensor_tensor(
                            out=xn[:], in0=xn[:], in1=lng_t[:], op=ALU.mult
                        )
                        nc.vector.tensor_tensor(
                            out=xn[:], in0=xn[:], in1=lnb_t[:], op=ALU.add
                        )
                        r = wk.tile([P, HID], bf16, tag="por")
                        nc.scalar.activation(r[:], xn[:], AF.Relu)
                        if li >= 1:
                            hres = wk.tile([P, HID], bf16, tag="pores")
                            nc.sync.dma_start(
                                out=hres[:], in_=h_src[li][b * P : (b + 1) * P, :]
                            )
                            nc.vector.tensor_add(r[:], r[:], hres[:])
                        if not last:
                            nc.sync.dma_start(
                                out=h_dst[li][b * P : (b + 1) * P, :], in_=r[:]
                            )
                        else:
                            h0t = wk.tile([P, HID], bf16, tag="poh0")
                            nc.sync.dma_start(
                                out=h0t[:], in_=h0[b * P : (b + 1) * P, :]
                            )
                            nc.vector.tensor_add(r[:], r[:], h0t[:])
                            nc.vector.tensor_scalar_add(
                                r[:], r[:], mask_t[:, b : b + 1]
                            )
                            # ---- fused pooling: mean + running max
                            g = b // (nnb // GPC)
                            for fc in range(4):
                                mp = ptr.tile([P, GPC], f32, tag="mp")
                                nc.tensor.matmul(
                                    out=mp[:],
                                    lhsT=r[:, fc * P : (fc + 1) * P],
                                    rhs=ppool_t[:, b * GPC : (b + 1) * GPC],
                                    start=True,
                                    stop=True,
                                )
                                nc.vector.tensor_add(
                                    mean_sb[:, fc * GPC : (fc + 1) * GPC],
                                    mean_sb[:, fc * GPC : (fc + 1) * GPC],
                                    mp[:],
                                )
                                pst = ptr.tile([P, P], bf16, tag="tp")
                                nc.tensor.transpose(
                                    pst[:], r[:, fc * P : (fc + 1) * P], ident[:]
                                )
                                mx = stp.tile([P, 1], f32, tag="mx")
                                nc.vector.reduce_max(
                                    out=mx[:], in_=pst[:], axis=mybir.AxisListType.X
                                )
                                col = fc * GPC + g
                                nc.vector.tensor_tensor(
                                    out=gmax_sb[:, col : col + 1],
                                    in0=gmax_sb[:, col : col + 1],
                                    in1=mx[:],
                                    op=ALU.max,
                                )

            # ================= pooled MLP =================
            with tc.tile_pool(name="pool", bufs=1) as plp:
                gmask_t = plp.tile([P, GPC], f32, tag="gmask")
                nc.sync.dma_start(out=gmask_t[:], in_=gmask[:])

                gkt = []  # 8 k-tiles of gT: 4 mean + 4 max, each [P, GPC] bf16
                for fc in range(4):
                    t = plp.tile([P, GPC], bf16, tag=f"gmean{fc}")
                    nc.vector.tensor_copy(
                        out=t[:], in_=mean_sb[:, fc * GPC : (fc + 1) * GPC]
                    )
                    gkt.append(t)
                for fc in range(4):
                    mxm = plp.tile([P, GPC], bf16, tag=f"mxm{fc}")
                    nc.vector.tensor_tensor(
                        out=mxm[:],
                        in0=gmax_sb[:, fc * GPC : (fc + 1) * GPC],
                        in1=gmask_t[:],
                        op=ALU.mult,
                    )
                    gkt.append(mxm)

                # MLP (transposed): a0T = relu(Wp0^T gT + b)
                wp0t = []
                for kt in range(8):
                    t = plp.tile([P, HID], bf16, tag=f"wp0_{kt}")
                    nc.sync.dma_start(out=t[:], in_=Wp0[kt * P : (kt + 1) * P, :])
                    wp0t.append(t)
                bp0_t = plp.tile([P, 4], f32, tag="bp0")
                nc.sync.dma_start(out=bp0_t[:], in_=bp0T[:])
                a0 = []
                for mt in range(4):
                    ps = ptr.tile([P, GPC], f32, tag="mp")
                    for kt in range(8):
                        nc.tensor.matmul(
                            out=ps[:],
                            lhsT=wp0t[kt][:, mt * P : (mt + 1) * P],
                            rhs=gkt[kt][:],
                            start=(kt == 0),
                            stop=(kt == 7),
                        )
                    t = plp.tile([P, GPC], bf16, tag=f"a0_{mt}")
                    nc.scalar.activation(
                        t[:], ps[:], AF.Relu, bias=bp0_t[:, mt : mt + 1]
                    )
                    a0.append(t)
                wp1t = []
                for kt in range(4):
                    t = plp.tile([P, HID // 2], bf16, tag=f"wp1_{kt}")
                    nc.sync.dma_start(out=t[:], in_=Wp1[kt * P : (kt + 1) * P, :])
                    wp1t.append(t)
                bp1_t = plp.tile([P, 2], f32, tag="bp1")
                nc.sync.dma_start(out=bp1_t[:], in_=bp1T[:])
                a1 = []
                for mt in range(2):
                    ps = ptr.tile([P, GPC], f32, tag="mp")
                    for kt in range(4):
                        nc.tensor.matmul(
                            out=ps[:],
                            lhsT=wp1t[kt][:, mt * P : (mt + 1) * P],
                            rhs=a0[kt][:],
                            start=(kt == 0),
                            stop=(kt == 3),
                        )
                    t = plp.tile([P, GPC], bf16, tag=f"a1_{mt}")
                    nc.scalar.activation(
                        t[:], ps[:], AF.Identity, bias=bp1_t[:, mt : mt + 1]
                    )
                    a1.append(t)
                wc0t = []
                for kt in range(2):
                    t = plp.tile([P, HID // 4], bf16, tag=f"wc0_{kt}")
                    nc.sync.dma_start(out=t[:], in_=Wc0[kt * P : (kt + 1) * P, :])
                    wc0t.append(t)
                bc0_t = plp.tile([P, 1], f32, tag="bc0")
                nc.sync.dma_start(out=bc0_t[:], in_=bc0T[:])
                ps = ptr.tile([P, GPC], f32, tag="mp")
                for kt in range(2):
                    nc.tensor.matmul(
                        out=ps[:],
                        lhsT=wc0t[kt][:],
                        rhs=a1[kt][:],
                        start=(kt == 0),
                        stop=(kt == 1),
                    )
                a2 = plp.tile([P, GPC], bf16, tag="a2")
                nc.scalar.activation(a2[:], ps[:], AF.Relu, bias=bc0_t[:, 0:1])
                wc1t = plp.tile([P, NCLS], bf16, tag="wc1")
                nc.sync.dma_start(out=wc1t[:], in_=Wc1[:])
                bc1_t = plp.tile([NCLS, 1], f32, tag="bc1")
                nc.sync.dma_start(out=bc1_t[:], in_=bc1[:])
                ps2 = ptr.tile([NCLS, GPC], f32, tag="mp")
                nc.tensor.matmul(
                    out=ps2[:], lhsT=wc1t[:], rhs=a2[:], start=True, stop=True
                )
                ot = plp.tile([NCLS, GPC], f32, tag="ot")
                nc.scalar.activation(ot[:], ps2[:], AF.Identity, bias=bc1_t[:, 0:1])
                nc.sync.dma_start(out=out[:], in_=ot[:])

    nc.compile()
    return nc


def make_in_maps(shared, per_core):
    maps = []
    for c in range(len(per_core)):
        m = dict(shared)
        m.update(per_core[c])
        m = {k: np.ascontiguousarray(v) for k, v in m.items()}
        maps.append(m)
    return maps


def kernel(**inputs) -> np.ndarray:
    sched, shared, per_core = preprocess(inputs)
    nc = build(sched, NCORES)
    in_maps = make_in_maps(shared, per_core)
    res = run_bass_kernel_spmd(nc, in_maps, list(range(NCORES)))
    full = np.zeros((NG, NCLS), np.float32)
    for c in range(NCORES):
        full[c * GPC : (c + 1) * GPC, :] = res.results[c]["out"].T
    return full
